# revision 1
# baseline (speedup 1.0000x reference)
"""CortexIIBlock TRN2 Bass kernel — 8-core data-parallel over (batch, seq-half).

Layout: activations transposed [feature, time] on-chip. All matmuls fp32r
(11-bit mantissa, fp32 accumulate). Depthwise causal convs = shifted
scalar_tensor_tensor FMAs on DVE. RMSNorm partition-reductions via
ones-matmul on PE; per-row scales broadcast via K=1 rank-1 matmuls.

Per core: T = 128 history + 2048 payload rows. History rows are the
previous 128 rows of the same sequence (zeros at sequence start); only
the up-projection "val" half is computed for them (conv lookback <= 6).
"""
import numpy as np

D = 1024
DFF = 4096
B = 4
S = 4096
H = 128          # history rows per shard
R = 2048         # payload rows per shard
T = H + R        # 2176
NCT = D // 128   # 8 channel tiles
NB = 4           # payload blocks of 512
BN = 512
EPS = 1e-6

_CACHE = {}


def _build():
    import concourse.bacc as bacc
    import concourse.mybir as mybir
    import concourse.tile as tile

    F32 = mybir.dt.float32
    F32R = mybir.dt.float32r
    BF16 = mybir.dt.bfloat16
    AF = mybir.ActivationFunctionType
    MUL = mybir.AluOpType.mult
    ADD = mybir.AluOpType.add

    nc = bacc.Bacc(None, target_bir_lowering=False)
    _lp = nc.allow_low_precision(reason="fp32r (11-bit mantissa) rounding is intentional")
    _lp.__enter__()

    xT_d = nc.dram_tensor("xT", [D, T], F32R, kind="ExternalInput")
    up_d = nc.dram_tensor("up_sb", [16, 128, D], F32R, kind="ExternalInput")
    down_d = nc.dram_tensor("down_sb", [8, 128, D], F32R, kind="ExternalInput")
    wg_d = nc.dram_tensor("wg_sb", [32, 128, D], F32R, kind="ExternalInput")
    wu_d = nc.dram_tensor("wu_sb", [32, 128, D], F32R, kind="ExternalInput")
    wo_d = nc.dram_tensor("wo_sb", [8, 128, DFF], F32R, kind="ExternalInput")
    sg_d = nc.dram_tensor("sg_sb", [NCT, 128, 3], F32R, kind="ExternalInput")
    ln1_d = nc.dram_tensor("ln1_sb", [NCT, 128, 1], F32, kind="ExternalInput")
    ln2_d = nc.dram_tensor("ln2_sb", [NCT, 128, 1], F32, kind="ExternalInput")
    taps_d = nc.dram_tensor("taps_sb", [NCT, 128, 15], F32, kind="ExternalInput")
    ones128_d = nc.dram_tensor("ones128", [128, 1], F32R, kind="ExternalInput")
    one1_d = nc.dram_tensor("one1", [1, 128], F32R, kind="ExternalInput")
    one11_d = nc.dram_tensor("one11", [1, 1], F32R, kind="ExternalInput")
    yT_d = nc.dram_tensor("yT", [D, R], F32, kind="ExternalOutput")

    with tile.TileContext(nc) as tc:
        with (
            tc.tile_pool(name="const", bufs=1) as cpool,
            tc.tile_pool(name="x2p", bufs=1, space="DRAM") as x2pool,
        ):
            # constants
            ones128 = cpool.tile([128, 1], F32R, tag="c_ones", name="c_ones")
            nc.sync.dma_start(ones128[:], ones128_d[:])
            one1 = cpool.tile([1, 128], F32R, tag="c_one1", name="c_one1")
            nc.sync.dma_start(one1[:], one1_d[:])
            one11 = cpool.tile([1, 1], F32R, tag="c_one11", name="c_one11")
            nc.sync.dma_start(one11[:], one11_d[:])
            eps_t = cpool.tile([1, 1], F32, tag="c_eps", name="c_eps")
            nc.vector.memset(eps_t[:], EPS)
            sg_t = cpool.tile([128, NCT, 3], F32R, tag="c_sg", name="c_sg")
            for c in range(NCT):
                nc.sync.dma_start(sg_t[:, c, :], sg_d[c])
            ln1_t = cpool.tile([128, NCT], F32, tag="c_ln1", name="c_ln1")
            ln2_t = cpool.tile([128, NCT], F32, tag="c_ln2", name="c_ln2")
            for c in range(NCT):
                nc.sync.dma_start(ln1_t[:, c:c + 1], ln1_d[c])
                nc.sync.dma_start(ln2_t[:, c:c + 1], ln2_d[c])
            taps_t = cpool.tile([128, NCT, 15], F32, tag="c_taps", name="c_taps")
            for c in range(NCT):
                nc.sync.dma_start(taps_t[:, c, :], taps_d[c])

            # post-mixer residual stream x2, staged in DRAM
            x2d = x2pool.tile([D, R], F32, tag="x2d", name="x2d")

            # ---------------- mixer ----------------
            prev_val = None
            with (
                tc.tile_pool(name="valp", bufs=2) as vpool,
                tc.tile_pool(name="mx", bufs=2) as mx,
                tc.tile_pool(name="wmix", bufs=3) as wmx,
                tc.tile_pool(name="psA", bufs=2, space="PSUM") as psA,
                tc.tile_pool(name="psB", bufs=2, space="PSUM") as psB,
                tc.tile_pool(name="pssm", bufs=2, space="PSUM") as pssm,
            ):
                for bi in range(NB + 1):
                    hist = bi == 0
                    N = H if hist else BN
                    c0 = 0 if hist else H + (bi - 1) * BN

                    xb = []
                    for c in range(NCT):
                        t_ = mx.tile([128, BN], F32R, tag=f"xb{c}", name=f"xb{c}", bufs=2)
                        nc.sync.dma_start(t_[:, :N], xT_d[c * 128:(c + 1) * 128, c0:c0 + N])
                        xb.append(t_)

                    # --- rmsnorm: msum = sum_d x^2 ---
                    msum = pssm.tile([1, BN], F32, tag="msum", name="msum", bufs=1)
                    for c in range(NCT):
                        sq = mx.tile([128, BN], F32R, tag="sq", name="sq")
                        nc.scalar.activation(sq[:, :N], xb[c][:, :N].bitcast(F32), AF.Square)
                        nc.tensor.matmul(msum[:, :N], ones128[:], sq[:, :N],
                                         start=(c == 0), stop=(c == NCT - 1))
                    sd = mx.tile([1, BN], F32, tag="sd", name="sd")
                    nc.scalar.activation(sd[:, :N], msum[:, :N], AF.Sqrt,
                                         bias=eps_t[:], scale=1.0 / D)
                    rstd = mx.tile([1, BN], F32R, tag="rstd", name="rstd")
                    nc.vector.reciprocal(rstd[:, :N], sd[:, :N])
                    rsb_ps = pssm.tile([128, BN], F32, tag="pbc", name="rsbp", bufs=1)
                    nc.tensor.matmul(rsb_ps[:, :N], one1[:], rstd[:, :N],
                                     start=True, stop=True)
                    rsb = mx.tile([128, BN], F32, tag="rsb", name="rsb")
                    nc.scalar.copy(rsb[:, :N], rsb_ps[:, :N])

                    # h = x * rstd * ln1w   (per c-tile, one fused DVE op)
                    hT = []
                    for c in range(NCT):
                        h_ = mx.tile([128, BN], F32R, tag=f"h{c}", name=f"h{c}", bufs=2)
                        nc.vector.scalar_tensor_tensor(
                            out=h_[:, :N], in0=xb[c][:, :N].bitcast(F32),
                            scalar=ln1_t[:, c:c + 1],
                            in1=rsb[:, :N], op0=MUL, op1=MUL)
                        hT.append(h_)

                    # --- val half of up-projection (m 8..15) ---
                    val = []
                    for c in range(NCT):
                        v_ = vpool.tile([128, 6 + BN], F32, tag=f"val{c}", name=f"val{c}")
                        val.append(v_)
                        if not hist:
                            nc.vector.tensor_copy(v_[:, 0:6], prev_val[c][:, (H if bi == 1 else BN):(H if bi == 1 else BN) + 6])
                    for m in range(NCT):
                        wt = wmx.tile([128, D], F32R, tag="wmix", name="wmix")
                        nc.sync.dma_start(wt[:], up_d[8 + m])
                        pv = psA.tile([128, BN], F32, tag="pmm", name="pval")
                        for k in range(NCT):
                            nc.tensor.matmul(pv[:, :N], wt[:, k * 128:(k + 1) * 128],
                                             hT[k][:, :N], start=(k == 0), stop=(k == NCT - 1))
                        nc.scalar.copy(val[m][:, 6:6 + N], pv[:, :N])

                    if hist:
                        prev_val = val
                        continue

                    # --- scale gates: sg = softmax(h @ sgw.T), per-row j ---
                    ej = []
                    for j in range(3):
                        pj = pssm.tile([1, BN], F32, tag="psg", name="psg", bufs=1)
                        for k in range(NCT):
                            nc.tensor.matmul(pj[:, :N], sg_t[:, k, j:j + 1], hT[k][:, :N],
                                             start=(k == 0), stop=(k == NCT - 1))
                        e_ = mx.tile([1, BN], F32R, tag=f"e{j}", name=f"e{j}")
                        nc.scalar.activation(e_[:, :N], pj[:, :N], AF.Exp)
                        ej.append(e_)
                    es = mx.tile([1, BN], F32, tag="es", name="es")
                    nc.vector.tensor_add(es[:, :N], ej[0][:, :N].bitcast(F32), ej[1][:, :N].bitcast(F32))
                    nc.vector.tensor_add(es[:, :N], es[:, :N], ej[2][:, :N].bitcast(F32))
                    erec = mx.tile([1, BN], F32, tag="erec", name="erec")
                    nc.vector.reciprocal(erec[:, :N], es[:, :N])
                    swb = []
                    for j in range(3):
                        swj = mx.tile([1, BN], F32R, tag="swj", name="swj")
                        nc.vector.tensor_mul(swj[:, :N], ej[j][:, :N].bitcast(F32), erec[:, :N])
                        pb_ = pssm.tile([128, BN], F32, tag="pbc", name="pswb", bufs=1)
                        nc.tensor.matmul(pb_[:, :N], one1[:], swj[:, :N], start=True, stop=True)
                        sb_ = mx.tile([128, BN], F32, tag=f"swb{j}", name=f"swb{j}")
                        nc.scalar.copy(sb_[:, :N], pb_[:, :N])
                        swb.append(sb_)

                    # --- gate (up m 0..7), conv, mix, z ---
                    zT = []
                    for c in range(NCT):
                        wt = wmx.tile([128, D], F32R, tag="wmix", name="wmix")
                        nc.sync.dma_start(wt[:], up_d[c])
                        pg = psA.tile([128, BN], F32, tag="pmm", name="pgate")
                        for k in range(NCT):
                            nc.tensor.matmul(pg[:, :N], wt[:, k * 128:(k + 1) * 128],
                                             hT[k][:, :N], start=(k == 0), stop=(k == NCT - 1))
                        gate = mx.tile([128, BN], F32, tag="gate", name="gate")
                        nc.scalar.activation(gate[:, :N], pg[:, :N], AF.Sigmoid)

                        v_ = val[c]
                        # c_fine (3 taps), c_med (5), c_coarse (7); tap jj order
                        convs = []
                        for (nt, base) in ((3, 0), (5, 3), (7, 8)):
                            ct_ = mx.tile([128, BN], F32, tag=f"cv{len(convs)}", name=f"cv{len(convs)}")
                            nc.vector.tensor_scalar_mul(
                                ct_[:, :N], v_[:, 6:6 + N], taps_t[:, c, base:base + 1])
                            for j in range(1, nt):
                                nc.vector.scalar_tensor_tensor(
                                    out=ct_[:, :N], in0=v_[:, 6 - j:6 - j + N],
                                    scalar=taps_t[:, c, base + j:base + j + 1],
                                    in1=ct_[:, :N], op0=MUL, op1=ADD)
                            convs.append(ct_)
                        acc = mx.tile([128, BN], F32, tag="acc", name="acc")
                        nc.vector.tensor_mul(acc[:, :N], convs[0][:, :N], swb[0][:, :N])
                        for j in (1, 2):
                            u_ = mx.tile([128, BN], F32, tag="mixu", name="mixu")
                            nc.vector.tensor_mul(u_[:, :N], convs[j][:, :N], swb[j][:, :N])
                            nc.vector.tensor_add(acc[:, :N], acc[:, :N], u_[:, :N])
                        z_ = mx.tile([128, BN], F32R, tag=f"z{c}", name=f"z{c}", bufs=1)
                        nc.vector.tensor_mul(z_[:, :N], acc[:, :N], gate[:, :N])
                        zT.append(z_)

                    # --- down projection + residual -> x2 ---
                    for m in range(NCT):
                        wt = wmx.tile([128, D], F32R, tag="wmix", name="wmix")
                        nc.sync.dma_start(wt[:], down_d[m])
                        pm = psB.tile([128, BN], F32, tag="pmix", name="pmix")
                        for k in range(NCT):
                            nc.tensor.matmul(pm[:, :N], wt[:, k * 128:(k + 1) * 128],
                                             zT[k][:, :N], start=(k == 0), stop=(k == NCT - 1))
                        x2b = mx.tile([128, BN], F32, tag="x2b", name="x2b")
                        nc.vector.tensor_add(
                            x2b[:, :N], xb[m][:, :N].bitcast(F32), pm[:, :N])
                        nc.sync.dma_start(
                            x2d[m * 128:(m + 1) * 128, c0 - H:c0 - H + N], x2b[:, :N])

                    prev_val = val

            # ---------------- FFN ----------------
            with (
                tc.tile_pool(name="fx", bufs=2) as fx,
                tc.tile_pool(name="pp", bufs=1) as pp,
                tc.tile_pool(name="wgu", bufs=6) as wgu,
                tc.tile_pool(name="wop", bufs=2) as wop,
                tc.tile_pool(name="psG", bufs=2, space="PSUM") as psG,
                tc.tile_pool(name="psU", bufs=2, space="PSUM") as psU,
                tc.tile_pool(name="psY", bufs=2, space="PSUM") as psY,
                tc.tile_pool(name="pss2", bufs=2, space="PSUM") as pss2,
            ):
                for rb in range(NB):
                    r0 = rb * BN
                    x2b = []
                    for c in range(NCT):
                        t_ = fx.tile([128, BN], F32, tag=f"x2r{c}", name=f"x2r{c}", bufs=1)
                        nc.sync.dma_start(t_[:], x2d[c * 128:(c + 1) * 128, r0:r0 + BN])
                        x2b.append(t_)
                    # rmsnorm(x2)
                    msum = pss2.tile([1, BN], F32, tag="msum2", name="msum2", bufs=1)
                    for c in range(NCT):
                        sq = fx.tile([128, BN], F32R, tag="sq2", name="sq2")
                        nc.scalar.activation(sq[:], x2b[c][:], AF.Square)
                        nc.tensor.matmul(msum[:], ones128[:], sq[:],
                                         start=(c == 0), stop=(c == NCT - 1))
                    sd = fx.tile([1, BN], F32, tag="sd2", name="sd2")
                    nc.scalar.activation(sd[:], msum[:], AF.Sqrt,
                                         bias=eps_t[:], scale=1.0 / D)
                    rstd = fx.tile([1, BN], F32R, tag="rstd2", name="rstd2")
                    nc.vector.reciprocal(rstd[:], sd[:])
                    rsb_ps = pss2.tile([128, BN], F32, tag="rsbp2", name="rsbp2", bufs=1)
                    nc.tensor.matmul(rsb_ps[:], one1[:], rstd[:], start=True, stop=True)
                    rsb = fx.tile([128, BN], F32, tag="rsb2", name="rsb2")
                    nc.scalar.copy(rsb[:], rsb_ps[:])
                    h2 = []
                    for c in range(NCT):
                        h_ = fx.tile([128, BN], F32R, tag=f"h2_{c}", name=f"h2_{c}", bufs=1)
                        nc.vector.scalar_tensor_tensor(
                            out=h_[:], in0=x2b[c][:],
                            scalar=ln2_t[:, c:c + 1],
                            in1=rsb[:], op0=MUL, op1=MUL)
                        h2.append(h_)

                    # g/u + silu + product -> p tiles
                    pT = []
                    for m in range(32):
                        wtg = wgu.tile([128, D], F32R, tag="wg", name="wg")
                        nc.sync.dma_start(wtg[:], wg_d[m])
                        pg = psG.tile([128, BN], F32, tag="pg", name="pg")
                        for k in range(NCT):
                            nc.tensor.matmul(pg[:], wtg[:, k * 128:(k + 1) * 128],
                                             h2[k][:], start=(k == 0), stop=(k == NCT - 1))
                        wtu = wgu.tile([128, D], F32R, tag="wu", name="wu")
                        nc.sync.dma_start(wtu[:], wu_d[m])
                        pu = psU.tile([128, BN], F32, tag="pu", name="pu")
                        for k in range(NCT):
                            nc.tensor.matmul(pu[:], wtu[:, k * 128:(k + 1) * 128],
                                             h2[k][:], start=(k == 0), stop=(k == NCT - 1))
                        tg = fx.tile([128, BN], F32, tag="tg", name="tg")
                        nc.scalar.activation(tg[:], pg[:], AF.Silu)
                        p_ = pp.tile([128, BN], F32R, tag=f"p{m}", name=f"p{m}")
                        nc.vector.tensor_mul(p_[:], tg[:], pu[:])
                        pT.append(p_)

                    # wo projection + residual -> out
                    for m in range(NCT):
                        wa = wop.tile([128, 2048], F32R, tag="woA", name="woA")
                        nc.sync.dma_start(wa[:], wo_d[m][:, 0:2048])
                        wb = wop.tile([128, 2048], F32R, tag="woB", name="woB")
                        nc.sync.dma_start(wb[:], wo_d[m][:, 2048:4096])
                        py = psY.tile([128, BN], F32, tag="py", name="py")
                        for k in range(32):
                            wt = wa if k < 16 else wb
                            ks = (k % 16) * 128
                            nc.tensor.matmul(py[:], wt[:, ks:ks + 128], pT[k][:],
                                             start=(k == 0), stop=(k == 31))
                        yo = fx.tile([128, BN], F32, tag="yo", name="yo")
                        nc.vector.tensor_add(yo[:], x2b[m][:], py[:])
                        nc.sync.dma_start(yT_d[m * 128:(m + 1) * 128, r0:r0 + BN], yo[:])

    if not nc.is_finalized():
        nc.finalize()
    return nc


def _host_prep(x, ln1_w, ln2_w, w_fine, w_medium, w_coarse, sg_w, up_w, down_w, wg, wu, wo):
    f = np.float32
    up_sb = np.ascontiguousarray(
        up_w.T.reshape(NCT, 128, 16, 128).transpose(2, 1, 0, 3).reshape(16, 128, D), f)
    down_sb = np.ascontiguousarray(
        down_w.T.reshape(NCT, 128, 8, 128).transpose(2, 1, 0, 3).reshape(8, 128, D), f)
    wg_sb = np.ascontiguousarray(
        wg.T.reshape(NCT, 128, 32, 128).transpose(2, 1, 0, 3).reshape(32, 128, D), f)
    wu_sb = np.ascontiguousarray(
        wu.T.reshape(NCT, 128, 32, 128).transpose(2, 1, 0, 3).reshape(32, 128, D), f)
    wo_sb = np.ascontiguousarray(
        wo.T.reshape(32, 128, 8, 128).transpose(2, 1, 0, 3).reshape(8, 128, DFF), f)
    sg_sb = np.ascontiguousarray(sg_w.T.reshape(NCT, 128, 3), f)
    ln1_sb = np.ascontiguousarray(ln1_w.reshape(NCT, 128, 1), f)
    ln2_sb = np.ascontiguousarray(ln2_w.reshape(NCT, 128, 1), f)
    taps = np.zeros((NCT, 128, 15), f)
    for (w_, nt, base) in ((w_fine, 3, 0), (w_medium, 5, 3), (w_coarse, 7, 8)):
        for j in range(nt):
            taps[:, :, base + j] = w_[:, 0, nt - 1 - j].reshape(NCT, 128)
    shared = dict(up_sb=up_sb, down_sb=down_sb, wg_sb=wg_sb, wu_sb=wu_sb,
                  wo_sb=wo_sb, sg_sb=sg_sb, ln1_sb=ln1_sb, ln2_sb=ln2_sb,
                  taps_sb=taps,
                  ones128=np.ones((128, 1), f), one1=np.ones((1, 128), f),
                  one11=np.ones((1, 1), f))
    in_maps = []
    for core in range(8):
        b, half = core // 2, core % 2
        if half == 0:
            histx = np.zeros((H, D), f)
            pay = x[b, 0:R]
        else:
            histx = x[b, R - H:R]
            pay = x[b, R:S]
        xT = np.ascontiguousarray(np.concatenate([histx, pay], 0).T, f)
        in_maps.append({**shared, "xT": xT})
    return in_maps


def kernel(**inputs):
    from concourse.bass_utils import run_bass_kernel_spmd
    if "nc" not in _CACHE:
        _CACHE["nc"] = _build()
    nc = _CACHE["nc"]
    in_maps = _host_prep(**{k: np.asarray(v) for k, v in inputs.items()})
    res = run_bass_kernel_spmd(nc, in_maps, core_ids=list(range(8)))
    out = np.empty((B, S, D), np.float32)
    for core in range(8):
        b, half = core // 2, core % 2
        out[b, half * R:(half + 1) * R] = res.results[core]["yT"].T
    return out



# revision 4
# speedup vs baseline: 1.2208x; 1.2208x over previous
"""CortexIIBlock TRN2 Bass kernel v2 — fused per-block mixer+FFN pipeline.

8-core data-parallel over (batch, seq-half): each core owns 2048 sequence
positions (+16 history cols for the causal convs). All matmuls bf16 inputs
with fp32 PSUM accumulation. Per-block software pipeline: FFN matmuls of
block i-1 run on PE while DVE computes the depthwise convs of block i+1
and GpSimd does the softmax-weighted conv mixing. x2 residual stays in
SBUF (no DRAM round trip).
"""
import numpy as np

D = 1024
DFF = 4096
B = 4
S = 4096
H = 16           # history cols per shard (conv lookback <= 6, padded to 16)
R = 2048         # payload cols per shard
NCT = D // 128   # 8 channel tiles
NB = 4           # payload blocks
BN = 512
EPS = 1e-6

_CACHE = {}


def _build():
    import concourse.bacc as bacc
    import concourse.mybir as mybir
    import concourse.tile as tile

    F32 = mybir.dt.float32
    BF16 = mybir.dt.bfloat16
    AF = mybir.ActivationFunctionType
    MUL = mybir.AluOpType.mult
    ADD = mybir.AluOpType.add

    nc = bacc.Bacc(None, target_bir_lowering=False)
    _lp = nc.allow_low_precision(reason="bf16 matmuls/activations within tolerance")
    _lp.__enter__()

    xT_d = nc.dram_tensor("xT", [128, NCT, H + R], BF16, kind="ExternalInput")
    # grouped weights: pairs of m-tiles side by side [128, 2*D]
    up_d = nc.dram_tensor("up_g", [8, 128, 2 * D], BF16, kind="ExternalInput")     # g0..3 gate, g4..7 val
    down_d = nc.dram_tensor("down_g", [4, 128, 2 * D], BF16, kind="ExternalInput")
    wg_d = nc.dram_tensor("wg_g", [16, 128, 2 * D], BF16, kind="ExternalInput")
    wu_d = nc.dram_tensor("wu_g", [16, 128, 2 * D], BF16, kind="ExternalInput")
    wo_d = nc.dram_tensor("wo_sb", [8, 128, DFF], BF16, kind="ExternalInput")
    sg_d = nc.dram_tensor("sg_p", [128, NCT, 3], BF16, kind="ExternalInput")
    ln1_d = nc.dram_tensor("ln1_p", [128, NCT], F32, kind="ExternalInput")
    ln2_d = nc.dram_tensor("ln2_p", [128, NCT], F32, kind="ExternalInput")
    taps_d = nc.dram_tensor("taps_p", [128, NCT, 15], F32, kind="ExternalInput")
    ones128_d = nc.dram_tensor("ones128", [128, 1], BF16, kind="ExternalInput")
    one1_d = nc.dram_tensor("one1", [1, 128], BF16, kind="ExternalInput")
    yT_d = nc.dram_tensor("yT", [128, NCT, R], F32, kind="ExternalOutput")

    from contextlib import ExitStack
    with tile.TileContext(nc) as tc:
        with ExitStack() as stack:
            ep = stack.enter_context
            cpool = ep(tc.tile_pool(name="const", bufs=1))
            xp = ep(tc.tile_pool(name="xp", bufs=3))
            hp = ep(tc.tile_pool(name="hp", bufs=2))
            vp = ep(tc.tile_pool(name="vp", bufs=2))
            x2p = ep(tc.tile_pool(name="x2p", bufs=1))
            h2p = ep(tc.tile_pool(name="h2p", bufs=1))
            gp = ep(tc.tile_pool(name="gp", bufs=1))
            cvp = ep(tc.tile_pool(name="cvp", bufs=2))
            zp = ep(tc.tile_pool(name="zp", bufs=1))
            ppool = ep(tc.tile_pool(name="pp", bufs=1))
            sbp = ep(tc.tile_pool(name="sb", bufs=2))
            smp = ep(tc.tile_pool(name="sm", bufs=2))
            tgp = ep(tc.tile_pool(name="tg", bufs=3))
            yp = ep(tc.tile_pool(name="yp", bufs=2))
            sqp = ep(tc.tile_pool(name="sqq", bufs=3))
            wmx = ep(tc.tile_pool(name="wmix", bufs=4))
            wgup = ep(tc.tile_pool(name="wgu", bufs=2))
            wop = ep(tc.tile_pool(name="wop", bufs=3))
            psmm = ep(tc.tile_pool(name="psmm", bufs=4, space="PSUM"))
            psbc = ep(tc.tile_pool(name="psbc", bufs=2, space="PSUM"))
            psrd = ep(tc.tile_pool(name="psrd", bufs=2, space="PSUM"))
            # ---------------- constants ----------------
            ones128 = cpool.tile([128, 1], BF16, tag="c_ones", name="c_ones")
            nc.sync.dma_start(ones128[:], ones128_d[:])
            one1 = cpool.tile([1, 128], BF16, tag="c_one1", name="c_one1")
            nc.sync.dma_start(one1[:], one1_d[:])
            eps_t = cpool.tile([1, 1], F32, tag="c_eps", name="c_eps")
            nc.vector.memset(eps_t[:], EPS)
            xh = cpool.tile([128, NCT, H], BF16, tag="xh", name="xh")
            nc.sync.dma_start(xh[:], xT_d[:, :, 0:H])
            ln1_t = cpool.tile([128, NCT], F32, tag="c_ln1", name="c_ln1")
            nc.sync.dma_start(ln1_t[:], ln1_d[:])

            # ---------------- persistent per-block state ----------------
            xb = [None] * NB
            hT = [None] * NB
            val = [None] * NB
            gate = [None] * NB
            swb = [None] * NB
            z = [None] * NB
            x2 = [None] * NB
            h2 = [None] * NB
            pT = [None] * NB

            def rmsnorm(src, ln_t, tag, is_mixer):
                msum = psrd.tile([1, BN], F32, tag="msum", name=f"msum_{tag}")
                for c in range(NCT):
                    sq = sqp.tile([128, BN], BF16, tag="sq", name=f"sq_{tag}{c}")
                    nc.scalar.activation(sq[:], src[:, c, :], AF.Square)
                    nc.tensor.matmul(msum[:], ones128[:], sq[:],
                                     start=(c == 0), stop=(c == NCT - 1))
                # rstd = exp(-0.5*ln(ms/D + eps)) — all on Act, no DVE dependency
                sd = smp.tile([1, BN], F32, tag="sd", name=f"sd_{tag}")
                nc.scalar.activation(sd[:], msum[:], AF.Ln,
                                     bias=eps_t[:], scale=1.0 / D)
                rstd = smp.tile([1, BN], BF16, tag="rstd", name=f"rstd_{tag}")
                nc.scalar.activation(rstd[:], sd[:], AF.Exp, scale=-0.5)
                rsb_ps = psbc.tile([128, BN], F32, tag="pbc", name=f"rsbp_{tag}")
                nc.tensor.matmul(rsb_ps[:], one1[:], rstd[:], start=True, stop=True)
                rsb = sbp.tile([128, BN], BF16, tag="rsb", name=f"rsb_{tag}")
                nc.scalar.copy(rsb[:], rsb_ps[:])
                h_ = (hp if is_mixer else h2p).tile(
                    [128, NCT, BN], BF16, tag="h" if is_mixer else "h2",
                    name=f"h_{tag}")
                for c in range(NCT):
                    nc.vector.scalar_tensor_tensor(
                        out=h_[:, c, :], in0=src[:, c, :],
                        scalar=ln_t[:, c:c + 1], in1=rsb[:], op0=MUL, op1=MUL)
                return h_

            # ---------------- history mini-front (16 cols) ----------------
            msumh = psrd.tile([1, H], F32, tag="msum", name="msumh")
            for c in range(NCT):
                sqh = sqp.tile([128, H], BF16, tag="sqh", name=f"sqh{c}", bufs=2)
                nc.scalar.activation(sqh[:], xh[:, c, :], AF.Square)
                nc.tensor.matmul(msumh[:], ones128[:], sqh[:],
                                 start=(c == 0), stop=(c == NCT - 1))
            sdh = smp.tile([1, H], F32, tag="sdh", name="sdh", bufs=1)
            nc.scalar.activation(sdh[:], msumh[:], AF.Ln, bias=eps_t[:], scale=1.0 / D)
            rstdh = smp.tile([1, H], BF16, tag="rstdh", name="rstdh", bufs=1)
            nc.scalar.activation(rstdh[:], sdh[:], AF.Exp, scale=-0.5)
            rsbh_ps = psbc.tile([128, H], F32, tag="pbc", name="rsbph")
            nc.tensor.matmul(rsbh_ps[:], one1[:], rstdh[:], start=True, stop=True)
            rsbh = smp.tile([128, H], BF16, tag="rsbh", name="rsbh", bufs=1)
            nc.scalar.copy(rsbh[:], rsbh_ps[:])
            hh = cpool.tile([128, NCT, H], BF16, tag="hh", name="hh")
            for c in range(NCT):
                nc.vector.scalar_tensor_tensor(
                    out=hh[:, c, :], in0=xh[:, c, :],
                    scalar=ln1_t[:, c:c + 1], in1=rsbh[:], op0=MUL, op1=MUL)
            valh = []
            for g in range(4):
                wt = wmx.tile([128, 2 * D], BF16, tag="wmix", name=f"wvh{g}")
                nc.sync.dma_start(wt[:], up_d[4 + g])
                for j in range(2):
                    m = 2 * g + j
                    pvh = psbc.tile([128, H], F32, tag="pbc", name=f"pvh{m}")
                    for k in range(NCT):
                        nc.tensor.matmul(pvh[:], wt[:, j * D + k * 128:j * D + (k + 1) * 128],
                                         hh[:, k, :], start=(k == 0), stop=(k == NCT - 1))
                    vh = cpool.tile([128, H], BF16, tag=f"vh{m}", name=f"vh{m}")
                    nc.scalar.copy(vh[:], pvh[:])
                    valh.append(vh)

            sg_t = cpool.tile([128, NCT, 3], BF16, tag="c_sg", name="c_sg")
            nc.sync.dma_start(sg_t[:], sg_d[:])
            ln2_t = cpool.tile([128, NCT], F32, tag="c_ln2", name="c_ln2")
            nc.sync.dma_start(ln2_t[:], ln2_d[:])
            taps_t = cpool.tile([128, NCT, 15], F32, tag="c_taps", name="c_taps")
            nc.sync.dma_start(taps_t[:], taps_d[:])

            # ---------------- per-block pieces ----------------
            def norm_front(i):
                c0 = H + i * BN
                x_ = xp.tile([128, NCT, BN], BF16, tag="xb", name=f"xb{i}")
                nc.sync.dma_start(x_[:], xT_d[:, :, c0:c0 + BN])
                xb[i] = x_
                hT[i] = rmsnorm(x_, ln1_t, f"m{i}", True)

            def body_front(i):
                # scale gates: softmax over 3 per-row chains (partition offsets
                # must be multiples of 32, so keep each row at partition 0)
                ej = []
                for j in range(3):
                    pj = psrd.tile([1, BN], F32, tag="msum", name=f"psg{i}_{j}")
                    for k in range(NCT):
                        nc.tensor.matmul(pj[:], sg_t[:, k, j:j + 1], hT[i][:, k, :],
                                         start=(k == 0), stop=(k == NCT - 1))
                    e_ = smp.tile([1, BN], BF16, tag=f"e{j}", name=f"e{i}_{j}", bufs=1)
                    nc.scalar.activation(e_[:], pj[:], AF.Exp)
                    ej.append(e_)
                es = smp.tile([1, BN], BF16, tag="es", name=f"es{i}")
                nc.vector.tensor_add(es[:], ej[0][:], ej[1][:])
                nc.vector.tensor_add(es[:], es[:], ej[2][:])
                erec = smp.tile([1, BN], BF16, tag="erec", name=f"erec{i}")
                nc.vector.reciprocal(erec[:], es[:])
                sw_ = []
                for j in range(3):
                    swj = smp.tile([1, BN], BF16, tag="swj", name=f"swj{i}_{j}")
                    nc.vector.tensor_mul(swj[:], ej[j][:], erec[:])
                    pb_ = psbc.tile([128, BN], F32, tag="pbc", name=f"pswb{i}_{j}")
                    nc.tensor.matmul(pb_[:], one1[:], swj[:], start=True, stop=True)
                    sb_ = sbp.tile([128, BN], BF16, tag=f"swb{j}", name=f"swb{i}_{j}", bufs=1)
                    nc.scalar.copy(sb_[:], pb_[:])
                    sw_.append(sb_)
                swb[i] = sw_

                # val half of up projection (groups 4..7)
                vtiles = []
                for m in range(NCT):
                    v_ = vp.tile([128, H + BN], BF16, tag=f"val{m}", name=f"val{i}_{m}")
                    vtiles.append(v_)
                val[i] = vtiles
                for g in range(4):
                    wt = wmx.tile([128, 2 * D], BF16, tag="wmix", name=f"wv{i}_{g}")
                    nc.sync.dma_start(wt[:], up_d[4 + g])
                    for j in range(2):
                        m = 2 * g + j
                        pv = psmm.tile([128, BN], F32, tag="pmm", name=f"pval{i}_{m}")
                        for k in range(NCT):
                            nc.tensor.matmul(pv[:], wt[:, j * D + k * 128:j * D + (k + 1) * 128],
                                             hT[i][:, k, :], start=(k == 0), stop=(k == NCT - 1))
                        nc.scalar.copy(vtiles[m][:, H:H + BN], pv[:])
                        if i == 0:
                            nc.vector.tensor_copy(vtiles[m][:, 0:H], valh[m][:])
                        else:
                            nc.vector.tensor_copy(vtiles[m][:, 0:H], val[i - 1][m][:, BN:BN + H])

                # gate half of up projection (groups 0..3)
                g_ = gp.tile([128, NCT, BN], BF16, tag="gate", name=f"gate{i}")
                gate[i] = g_
                for g in range(4):
                    wt = wmx.tile([128, 2 * D], BF16, tag="wmix", name=f"wgm{i}_{g}")
                    nc.sync.dma_start(wt[:], up_d[g])
                    for j in range(2):
                        m = 2 * g + j
                        pg = psmm.tile([128, BN], F32, tag="pmm", name=f"pgate{i}_{m}")
                        for k in range(NCT):
                            nc.tensor.matmul(pg[:], wt[:, j * D + k * 128:j * D + (k + 1) * 128],
                                             hT[i][:, k, :], start=(k == 0), stop=(k == NCT - 1))
                        nc.scalar.activation(g_[:, m, :], pg[:], AF.Sigmoid)

                # convs + softmax-weighted mix.
                # Steady state: convs on DVE, mix on GpSimd, both at low priority
                # (pure gap-filler; deadline is down(i) one iteration later).
                # Block 0 is the pipeline prologue and on the critical path, so
                # split the work across both engines at normal priority instead.
                z_ = zp.tile([128, NCT, BN], BF16, tag="z", name=f"z{i}")
                z[i] = z_
                lowp = None
                if i > 0:
                    lowp = tc.high_priority(offset=-10_000_000)
                    lowp.__enter__()
                for c in range(NCT):
                    conv_eng = nc.vector
                    mix_eng = nc.vector if (i == 0 and c >= 6) else nc.gpsimd
                    v_ = vtiles[c]
                    convs = []
                    for (nt, base) in ((3, 0), (5, 3), (7, 8)):
                        ct_ = cvp.tile([128, BN], BF16, tag=f"cv{len(convs)}",
                                       name=f"cv{i}_{c}_{len(convs)}")
                        conv_eng.tensor_scalar_mul(
                            ct_[:], v_[:, H:H + BN], taps_t[:, c, base:base + 1])
                        for j in range(1, nt):
                            conv_eng.scalar_tensor_tensor(
                                out=ct_[:], in0=v_[:, H - j:H - j + BN],
                                scalar=taps_t[:, c, base + j:base + j + 1],
                                in1=ct_[:], op0=MUL, op1=ADD)
                        convs.append(ct_)
                    acc = cvp.tile([128, BN], BF16, tag="acc", name=f"acc{i}_{c}")
                    mix_eng.tensor_mul(acc[:], convs[0][:], sw_[0][:])
                    for j in (1, 2):
                        u_ = cvp.tile([128, BN], BF16, tag="mixu", name=f"mixu{i}_{c}")
                        mix_eng.tensor_mul(u_[:], convs[j][:], sw_[j][:])
                        mix_eng.tensor_add(acc[:], acc[:], u_[:])
                    mix_eng.tensor_mul(z_[:, c, :], acc[:], g_[:, c, :])
                if lowp is not None:
                    lowp.__exit__(None, None, None)

            def down_block(i):
                x2_ = x2p.tile([128, NCT, BN], BF16, tag="x2", name=f"x2_{i}")
                x2[i] = x2_
                for g in range(4):
                    wt = wmx.tile([128, 2 * D], BF16, tag="wmix", name=f"wd{i}_{g}")
                    nc.sync.dma_start(wt[:], down_d[g])
                    for j in range(2):
                        m = 2 * g + j
                        pm = psmm.tile([128, BN], F32, tag="pmm", name=f"pmix{i}_{m}")
                        for k in range(NCT):
                            nc.tensor.matmul(pm[:], wt[:, j * D + k * 128:j * D + (k + 1) * 128],
                                             z[i][:, k, :], start=(k == 0), stop=(k == NCT - 1))
                        nc.vector.tensor_add(x2_[:, m, :], xb[i][:, m, :], pm[:])

            def ffn_norm(i):
                h2[i] = rmsnorm(x2[i], ln2_t, f"f{i}", False)

            def ffn_gup(i):
                p_ = []
                for g in range(16):
                    wtg = wgup.tile([128, 2 * D], BF16, tag="wg", name=f"wgt{i}_{g}")
                    nc.sync.dma_start(wtg[:], wg_d[g])
                    wtu = wgup.tile([128, 2 * D], BF16, tag="wu", name=f"wut{i}_{g}")
                    nc.sync.dma_start(wtu[:], wu_d[g])
                    for j in range(2):
                        m = 2 * g + j
                        pg = psmm.tile([128, BN], F32, tag="pmm", name=f"pg{i}_{m}")
                        for k in range(NCT):
                            nc.tensor.matmul(pg[:], wtg[:, j * D + k * 128:j * D + (k + 1) * 128],
                                             h2[i][:, k, :], start=(k == 0), stop=(k == NCT - 1))
                        pu = psmm.tile([128, BN], F32, tag="pmm", name=f"pu{i}_{m}")
                        for k in range(NCT):
                            nc.tensor.matmul(pu[:], wtu[:, j * D + k * 128:j * D + (k + 1) * 128],
                                             h2[i][:, k, :], start=(k == 0), stop=(k == NCT - 1))
                        tg = tgp.tile([128, BN], BF16, tag="tg", name=f"tg{i}_{m}")
                        nc.scalar.activation(tg[:], pg[:], AF.Silu)
                        pt = ppool.tile([128, BN], BF16, tag=f"p{m}", name=f"p{i}_{m}")
                        nc.vector.tensor_mul(pt[:], tg[:], pu[:])
                        p_.append(pt)
                pT[i] = p_

            def ffn_out(i):
                for m in range(NCT):
                    wa = wop.tile([128, DFF // 2], BF16, tag="wo", name=f"woA{i}_{m}")
                    nc.sync.dma_start(wa[:], wo_d[m][:, 0:DFF // 2])
                    wb = wop.tile([128, DFF // 2], BF16, tag="wo", name=f"woB{i}_{m}")
                    nc.sync.dma_start(wb[:], wo_d[m][:, DFF // 2:DFF])
                    py = psmm.tile([128, BN], F32, tag="pmm", name=f"py{i}_{m}")
                    for k in range(32):
                        wt = wa if k < 16 else wb
                        ks = (k % 16) * 128
                        nc.tensor.matmul(py[:], wt[:, ks:ks + 128], pT[i][k][:],
                                         start=(k == 0), stop=(k == 31))
                    yo = yp.tile([128, BN], F32, tag="yo", name=f"yo{i}_{m}")
                    nc.vector.tensor_add(yo[:], x2[i][:, m, :], py[:])
                    nc.sync.dma_start(yT_d[:, m, i * BN:(i + 1) * BN], yo[:])

            # ---------------- schedule ----------------
            norm_front(0)
            norm_front(1)
            body_front(0)
            norm_front(2)
            body_front(1)
            down_block(0)
            for i in range(1, NB):
                ffn_norm(i - 1)
                if i + 2 < NB:
                    norm_front(i + 2)
                if i + 1 < NB:
                    body_front(i + 1)
                ffn_gup(i - 1)
                ffn_out(i - 1)
                down_block(i)
            ffn_norm(NB - 1)
            ffn_gup(NB - 1)
            ffn_out(NB - 1)

    if not nc.is_finalized():
        nc.finalize()
    return nc


def _host_prep(x, ln1_w, ln2_w, w_fine, w_medium, w_coarse, sg_w, up_w, down_w, wg, wu, wo):
    import ml_dtypes
    f = np.float32
    bf = ml_dtypes.bfloat16

    def mtiles(w):  # [F, D] -> [F//128, 128, D] stationary tiles
        F_ = w.shape[0]
        return np.ascontiguousarray(
            w.T.reshape(NCT, 128, F_ // 128, 128).transpose(2, 1, 0, 3).reshape(F_ // 128, 128, D))

    def group2(t):  # [M,128,D] -> [M//2,128,2D]
        M = t.shape[0]
        return np.ascontiguousarray(
            t.reshape(M // 2, 2, 128, D).transpose(0, 2, 1, 3).reshape(M // 2, 128, 2 * D))

    up_g = group2(mtiles(up_w)).astype(bf)            # g0..3 gate, g4..7 val
    down_g = group2(mtiles(down_w)).astype(bf)
    wg_g = group2(mtiles(wg)).astype(bf)
    wu_g = group2(mtiles(wu)).astype(bf)
    wo_sb = np.ascontiguousarray(
        wo.T.reshape(32, 128, 8, 128).transpose(2, 1, 0, 3).reshape(8, 128, DFF)).astype(bf)
    sg_p = np.ascontiguousarray(
        sg_w.T.reshape(NCT, 128, 3).transpose(1, 0, 2)).astype(bf)      # [128, NCT, 3]
    ln1_p = np.ascontiguousarray(ln1_w.reshape(NCT, 128).T, f)          # [128, NCT]
    ln2_p = np.ascontiguousarray(ln2_w.reshape(NCT, 128).T, f)
    taps = np.zeros((NCT, 128, 15), f)
    for (w_, nt, base) in ((w_fine, 3, 0), (w_medium, 5, 3), (w_coarse, 7, 8)):
        for j in range(nt):
            taps[:, :, base + j] = w_[:, 0, nt - 1 - j].reshape(NCT, 128)
    taps_p = np.ascontiguousarray(taps.transpose(1, 0, 2))              # [128, NCT, 15]
    shared = dict(up_g=up_g, down_g=down_g, wg_g=wg_g, wu_g=wu_g,
                  wo_sb=wo_sb, sg_p=sg_p, ln1_p=ln1_p, ln2_p=ln2_p,
                  taps_p=taps_p,
                  ones128=np.ones((128, 1), bf), one1=np.ones((1, 128), bf))
    in_maps = []
    for core in range(8):
        b, half = core // 2, core % 2
        if half == 0:
            histx = np.zeros((H, D), f)
            pay = x[b, 0:R]
        else:
            histx = x[b, R - H:R]
            pay = x[b, R:S]
        xcat = np.concatenate([histx, pay], 0)        # [H+R, D]
        xTh = np.ascontiguousarray(
            xcat.reshape(H + R, NCT, 128).transpose(2, 1, 0)).astype(bf)
        in_maps.append({**shared, "xT": xTh})
    return in_maps


def kernel(**inputs):
    from concourse.bass_utils import run_bass_kernel_spmd
    if "nc" not in _CACHE:
        _CACHE["nc"] = _build()
    nc = _CACHE["nc"]
    in_maps = _host_prep(**{k: np.asarray(v) for k, v in inputs.items()})
    res = run_bass_kernel_spmd(nc, in_maps, core_ids=list(range(8)))
    out = np.empty((B, S, D), np.float32)
    for core in range(8):
        b, half = core // 2, core % 2
        yTh = res.results[core]["yT"]                 # [128, NCT, R]
        out[b, half * R:(half + 1) * R] = yTh.transpose(2, 1, 0).reshape(R, D)
    return out
